# revision 8
# baseline (speedup 1.0000x reference)
"""GAT (3-layer, 4-head) Trainium2 kernel — 8 NeuronCores, node-tile-parallel.

v2.  Key structure (host prep + device):
  - Add self-loops; relabel nodes: pick the 28672 highest-degree nodes as
    table-half A (device rows < 28672 = collective chunks 0-3), rest half B.
    Within each half sort nodes by (dA, dB) = (#A-sources, #B-sources) so the
    128-row tiles are (dA, dB)-homogeneous -> near-tight padded CSR.
  - Per node-tile padded CSR: each node's in-edge sources as slots on its
    partition; slots split A-half/B-half (dma_gather indices are int16).
    Table layout: [128 sentinel | 28672 A | 21504 B+pad | 128 sentinel];
    B gathers use base offset 28800.  Sentinel rows hold -1e4 so padded
    slots produce exp()->0 weights — no mask tensor needed.
  - Householder trick: rotate each head's 32-dim block so att_src becomes
    ||a||*e0.  Per-edge alpha_src is channel h*32+0 of the gathered rows;
    alpha_dst comes out of the phase-1 matmul as 4 extra columns.
  - Gathers are prepare_only SWDGE preps spread over 4 queues + trigger_dma:
    descriptor generation pipelines with the DMA drain and other engines.
  - Per layer: per-tile matmul h~ = xT_tile @ [W~^T | Vdst] -> write own
    slice -> chunked AllGather into per-layer bf16 table -> per-tile gather
    -> DVE e/lrelu + ACT exp-expand -> flat multiply + single f32-accum
    reduce -> 1/s normalize -> transpose + un-rotate matmul (bmat stationary,
    result lands feature-major) -> ACT relu straight into next layer's xT.
  - softmax max-subtraction replaced by constant shift C (softmax-invariant;
    e range ~|e|<30 for this distribution, exp is f32-safe).
"""

import numpy as np

N = 50000
E = 800000
F = 128          # features (= H*C)
H = 4
CH = 32
L = 3
NC = 8           # cores
NPAD = 50176     # 392 tiles * 128
TILES = NPAD // 128
CHT = 7          # tiles per collective chunk (7 chunks of 7 per core)
TPC = TILES // NC       # 49 tiles per core
RPC = NPAD // NC        # 6272 rows per core
NA = 28672              # nodes in table half A (device rows < NA)
SPLIT = 28800           # table row base of half B (= 128 + NA)
TBL = 50432             # 128 sentinel + NPAD + 128 sentinel
BSENT = NPAD - NA       # B-half sentinel relative index (21504)
CSHIFT = 20.0           # constant exp shift (softmax invariant)
SENTV = -1.0e4          # sentinel row value
FOUT = 32
NQ = 4                  # SWDGE queues


def _prep_graph(edge_index):
    """Relabel nodes and build per-core padded-CSR gather indices.

    Returns (rowof[N]->device row, gidx[NC,128,8*sumK] int16, piece table,
    K_A, K_B, col offsets).
    """
    src = np.concatenate([edge_index[0].astype(np.int64), np.arange(N)])
    dst = np.concatenate([edge_index[1].astype(np.int64), np.arange(N)])
    deg = np.bincount(dst, minlength=N)
    order0 = np.argsort(-deg, kind="stable")
    inA = np.zeros(N, bool)
    inA[order0[:NA]] = True
    srcA = inA[src]
    dA = np.bincount(dst[srcA], minlength=N)
    dB = np.bincount(dst[~srcA], minlength=N)

    def sort_half(nodes):
        key = (dA[nodes].astype(np.int64) << 20) + dB[nodes]
        return nodes[np.argsort(-key, kind="stable")]

    pos2node = np.concatenate([sort_half(order0[:NA]), sort_half(order0[NA:])])
    # position i -> device row (tile deal: t=i//128 global tile, c=t%8, k=t//8)
    ii = np.arange(NPAD)
    t = ii // 128
    p = ii % 128
    kk_ = t // NC
    cc_ = t % NC
    row_of_pos = ((kk_ // CHT) * (NC * CHT * 128) + cc_ * (CHT * 128)
                  + (kk_ % CHT) * 128 + p)
    rowof = np.empty(N, np.int64)
    rowof[pos2node] = row_of_pos[:N]
    # A-half positions (i < NA) land on device rows < NA by construction

    srcr = rowof[src]
    dstr = rowof[dst]
    o = np.argsort(dstr, kind="stable")
    srcr_s = srcr[o]
    dstr_s = dstr[o]
    cnt = np.bincount(dstr_s, minlength=NPAD)
    off = np.concatenate([[0], np.cumsum(cnt)])

    srcsA = [None] * NPAD
    srcsB = [None] * NPAD
    nA_ = np.zeros(NPAD, np.int64)
    nB_ = np.zeros(NPAD, np.int64)
    for r in range(NPAD):
        s = srcr_s[off[r]:off[r + 1]]
        a = s[s < NA] + 128           # absolute table rows (sentinel offset)
        b = s[s >= NA] - NA           # relative to SPLIT
        srcsA[r] = a
        srcsB[r] = b
        nA_[r] = len(a)
        nB_[r] = len(b)

    def row0(c, k):
        return (k // CHT) * (NC * CHT * 128) + c * (CHT * 128) + (k % CHT) * 128

    K_A = np.zeros(TPC, np.int64)
    K_B = np.zeros(TPC, np.int64)
    for c in range(NC):
        for k in range(TPC):
            r0 = row0(c, k)
            K_A[k] = max(K_A[k], nA_[r0:r0 + 128].max())
            K_B[k] = max(K_B[k], nB_[r0:r0 + 128].max())

    # pieces per tile: split A and B halves in two, queues rotate
    # piece = (slot_off, n_slots, is_b)
    pieces = []
    oi = np.zeros(TPC + 1, np.int64)
    for k in range(TPC):
        ka, kb = int(K_A[k]), int(K_B[k])
        pl = []
        for base, n, isb in ((0, ka, 0), (ka, kb, 1)):
            if n > 0:
                pl.append((base, n, isb))
        pieces.append(pl)
        oi[k + 1] = oi[k] + 8 * (ka + kb)

    sumK = int((K_A + K_B).sum())
    gidx = np.zeros((NC, 128, 8 * sumK), np.int16)
    for c in range(NC):
        for k in range(TPC):
            r0 = row0(c, k)
            ka, kb = int(K_A[k]), int(K_B[k])
            kk = ka + kb
            if kk == 0:
                continue
            lin = np.zeros(128 * kk, np.int64)
            # padding defaults: A slots -> 0 (sentinel), B slots -> BSENT
            lin[128 * ka:] = BSENT
            for pp in range(128):
                r = r0 + pp
                a = srcsA[r]
                b = srcsB[r]
                if len(a):
                    lin[pp + 128 * np.arange(len(a))] = a
                if len(b):
                    lin[pp + 128 * (ka + np.arange(len(b)))] = b
            # wrap each piece's index sub-list independently
            for (s0, n, isb) in pieces[k]:
                sub = lin[128 * s0:128 * (s0 + n)].astype(np.int16)
                wr = sub.reshape(-1, 16).T          # [16, 8*n]
                gidx[c, :, oi[k] + 8 * s0: oi[k] + 8 * (s0 + n)] = \
                    np.tile(wr, (8, 1))
    return rowof, gidx, pieces, K_A, K_B, oi


def _prep_weights(Ws, att_src, att_dst, conv_bias, Wf, bf):
    """Householder-rotated weights.  Returns wcat [L,128,132], bmat [L,128,128],
    anorm [L,4], wfT [128,32]."""
    assert np.allclose(conv_bias, 0.0) and np.allclose(bf, 0.0), \
        "bias assumed zero (spec fill=zeros)"
    eye = np.eye(CH, dtype=np.float64)
    wcat = np.zeros((L, F, F + H), np.float32)
    bmat = np.zeros((L, F, F), np.float32)
    anorm = np.zeros((L, H), np.float32)
    for l in range(L):
        W = Ws[l].astype(np.float64)            # [F, F] (H*C, F_in)
        Bfull = np.zeros((F, F))
        for h in range(H):
            a = att_src[l, h].astype(np.float64)
            na = np.linalg.norm(a)
            anorm[l, h] = na
            if na < 1e-12:
                R = eye.copy()
            else:
                v = a.copy()
                v[0] -= na
                nv = np.linalg.norm(v)
                R = eye - 2.0 * np.outer(v, v) / (nv * nv) if nv > 1e-12 else eye.copy()
            Bfull[h * CH:(h + 1) * CH, h * CH:(h + 1) * CH] = R
        Wt = Bfull @ W                           # rotated W
        vcols = np.zeros((F, H))
        for h in range(H):
            blk = np.zeros(F)
            blk[h * CH:(h + 1) * CH] = att_dst[l, h]
            vcols[:, h] = W.T @ blk
        wcat_l = np.concatenate([Wt.T, vcols], axis=1)
        bm_l = Bfull.copy()
        for h in range(H):
            na = float(anorm[l, h])
            if na > 1e-12:
                wcat_l[:, h * CH] *= na
                bm_l[h * CH, :] /= na
        wcat[l] = wcat_l.astype(np.float32)
        bmat[l] = bm_l.astype(np.float32)
    wfT = Wf.T.astype(np.float32)               # [F, FOUT]
    return wcat, bmat, anorm, wfT


def _rows_of_core(c):
    blocks = [j * (NC * CHT * 128) + c * (CHT * 128) + np.arange(CHT * 128)
              for j in range(TPC // CHT)]
    return np.concatenate(blocks)


def _golden_device(x_dev, gidx, pieces, K_A, K_B, oi, wcat, bmat, wfT):
    """Numpy mirror of the device computation (same layouts & dtype casts)."""
    import ml_dtypes
    bf16 = ml_dtypes.bfloat16
    f32 = np.float32

    # rebuild the full table index list per (c, k) from gidx
    x_bf = x_dev.astype(bf16)                   # [NPAD, F] device-row order
    for l in range(L):
        wcat_bf = wcat[l].astype(bf16)
        had = x_bf.astype(f32) @ wcat_bf.astype(f32)     # [NPAD, 132]
        table = np.full((TBL, F), SENTV, f32)
        table[128:128 + NPAD] = had[:, :F].astype(bf16).astype(f32)
        adst = had[:, F:F + H].astype(f32)               # [NPAD, 4]
        x_new = np.zeros((NPAD, F), f32)
        for c in range(NC):
            for k in range(TPC):
                r0 = ((k // CHT) * (NC * CHT * 128) + c * (CHT * 128)
                      + (k % CHT) * 128)
                ka, kb = int(K_A[k]), int(K_B[k])
                kk = ka + kb
                # reconstruct per-piece linear index lists
                lin = np.zeros(128 * kk, np.int64)
                for (s0, n, isb) in pieces[k]:
                    blk = gidx[c, :16, oi[k] + 8 * s0: oi[k] + 8 * (s0 + n)]
                    sub = blk.T.reshape(-1).astype(np.int64)
                    sub = np.where(sub < 0, sub + 65536, sub)
                    if isb:
                        sub = sub + SPLIT
                    lin[128 * s0:128 * (s0 + n)] = sub
                G = table[lin].reshape(kk, 128, F).transpose(1, 0, 2)
                G = G.astype(bf16)
                asrc = G[:, :, 0::CH].astype(f32)          # [128,kk,H]
                e = asrc + adst[r0:r0 + 128][:, None, :]
                e = np.maximum(e, 0.2 * e)
                w = np.exp(np.minimum(e - CSHIFT, 80.0)).astype(bf16)
                s = w.astype(f32).sum(axis=1)              # [128,H]
                rs = (1.0 / (s + 1e-16)).astype(f32)
                wex = w.reshape(128, kk, H, 1).repeat(CH, 3).reshape(128, kk, F)
                M = (G * wex).astype(bf16)
                orot = M.astype(f32).sum(axis=1)           # [128,F] f32 accum
                outn = (orot * rs.reshape(128, H, 1).repeat(CH, 2)
                        .reshape(128, F)).astype(bf16)
                xp = np.maximum(outn.astype(f32) @ bmat[l].astype(bf16).astype(f32), 0.0)
                x_new[r0:r0 + 128] = xp
        x_bf = x_new.astype(bf16)
    out = x_bf.astype(f32) @ wfT.astype(bf16).astype(f32)  # [NPAD, FOUT]
    return out


def _host_prep(inputs):
    x = inputs["x"]
    edge_index = inputs["edge_index"]
    rowof, gidx, pieces, K_A, K_B, oi = _prep_graph(edge_index)
    wcat, bmat, anorm, wfT = _prep_weights(
        inputs["Ws"], inputs["att_src"], inputs["att_dst"],
        inputs["conv_bias"], inputs["Wf"], inputs["bf"])
    x_dev = np.zeros((NPAD, F), np.float32)
    x_dev[rowof] = x
    return dict(rowof=rowof, gidx=gidx, pieces=pieces,
                K_A=K_A, K_B=K_B, oi=oi, wcat=wcat, bmat=bmat,
                anorm=anorm, wfT=wfT, x_dev=x_dev)


def kernel_golden(**inputs):
    """Pure-numpy end-to-end (for validation)."""
    pp = _host_prep(inputs)
    out_dev = _golden_device(
        pp["x_dev"], pp["gidx"], pp["pieces"], pp["K_A"], pp["K_B"],
        pp["oi"], pp["wcat"], pp["bmat"], pp["wfT"])
    return out_dev[pp["rowof"]].astype(np.float32)


def _ap_view(base_ap, free_dims):
    """AP with same tensor/partition dim but custom free dims [(stride, n), ...]."""
    import concourse.bass as bass
    return bass.AP(
        tensor=base_ap.tensor,
        offset=base_ap.offset,
        ap=[list(base_ap.ap[0])] + [[s, n] for s, n in free_dims],
    )


def _build_bass(pp, repeats=1):
    import sys
    if "/opt/trn_rl_repo" not in sys.path:
        sys.path.insert(0, "/opt/trn_rl_repo")
    from contextlib import ExitStack
    import concourse.tile as tile
    from concourse import bass, mybir
    from concourse.bacc import Bacc
    from concourse.masks import make_identity

    K_A, K_B, oi, pieces = pp["K_A"], pp["K_B"], pp["oi"], pp["pieces"]
    K = K_A + K_B
    sumK = int(K.sum())
    NI = 8 * sumK
    f32 = mybir.dt.float32
    bf = mybir.dt.bfloat16
    i16 = mybir.dt.int16
    AX = mybir.AxisListType
    ALU = mybir.AluOpType
    ACTF = mybir.ActivationFunctionType

    nc = Bacc(None, num_devices=NC)
    xT_p = nc.declare_dram_parameter("xT", [F, RPC], f32, isOutput=False)
    gidx_p = nc.declare_dram_parameter("gidx", [128, NI], i16, isOutput=False)
    wcat_p = nc.declare_dram_parameter("wcat", [L, F, F + H], f32, isOutput=False)
    bmat_p = nc.declare_dram_parameter("bmat", [L, F, F], f32, isOutput=False)
    wfT_p = nc.declare_dram_parameter("wfT", [F, FOUT], f32, isOutput=False)
    out_p = nc.declare_dram_parameter("out", [RPC, FOUT], f32, isOutput=True)

    hslice = nc.dram_tensor("hslice", [RPC, F], bf)
    tables = [nc.dram_tensor(f"table{l}", [TBL, F], bf, addr_space="Shared")
              for l in range(L)]

    ctx = ExitStack()
    with ctx:
        tc = ctx.enter_context(tile.TileContext(nc))
        persist = ctx.enter_context(tc.tile_pool(name="persist", bufs=1))
        wpool = ctx.enter_context(tc.tile_pool(name="wpool", bufs=1))
        gpool = ctx.enter_context(tc.tile_pool(name="gpool", bufs=3))
        mpool = ctx.enter_context(tc.tile_pool(name="mpool", bufs=2))
        wxpool = ctx.enter_context(tc.tile_pool(name="wxpool", bufs=2))
        spool = ctx.enter_context(tc.tile_pool(name="spool", bufs=3))
        pspool = ctx.enter_context(tc.tile_pool(name="pspool", bufs=2, space="PSUM"))
        pspool2 = ctx.enter_context(tc.tile_pool(name="pspool2", bufs=2, space="PSUM"))

        ident = persist.tile([128, 128], bf, name="ident")
        make_identity(nc, ident[:])
        nidx_reg = nc.gpsimd.alloc_register("nidx")
        negc = persist.tile([128, 1], f32, name="negc")
        nc.vector.memset(negc[:], -CSHIFT)
        zero_b = persist.tile([128, 1], f32, name="zero_b")
        nc.vector.memset(zero_b[:], 0.0)

        # sentinel rows of each table
        sent = persist.tile([128, F], bf, name="sent")
        nc.vector.memset(sent[:], SENTV)
        for tbl in tables:
            nc.sync.dma_start(out=tbl[0:128, :], in_=sent[:])
            nc.sync.dma_start(out=tbl[TBL - 128:TBL, :], in_=sent[:])

        # resident weights / constants
        wcat_sb, bmat_sb = [], []
        for l in range(L):
            wc_f = spool.tile([F, F + H], f32, name=f"wc_f{l}", tag="wcf")
            nc.sync.dma_start(out=wc_f[:], in_=wcat_p[l])
            wc = wpool.tile([F, F + H], bf, name=f"wcat{l}")
            nc.vector.tensor_copy(out=wc[:], in_=wc_f[:])
            wcat_sb.append(wc)
            bm_f = spool.tile([F, F], f32, name=f"bm_f{l}", tag="bmf")
            nc.sync.dma_start(out=bm_f[:], in_=bmat_p[l])
            bm = wpool.tile([F, F], bf, name=f"bmat{l}")
            nc.vector.tensor_copy(out=bm[:], in_=bm_f[:])
            bmat_sb.append(bm)
        wfT_f = spool.tile([F, FOUT], f32, name="wfT_f", tag="wcf")
        nc.sync.dma_start(out=wfT_f[:], in_=wfT_p[:])
        wfT_sb = wpool.tile([F, FOUT], bf, name="wfT")
        nc.vector.tensor_copy(out=wfT_sb[:], in_=wfT_f[:])

        # resident graph indices
        gidx_sb = persist.tile([128, NI], i16, name="gidx_sb")
        nc.sync.dma_start(out=gidx_sb[:], in_=gidx_p[:])

        xT_tiles = [persist.tile([128, 128], bf, name=f"xT{k}") for k in range(TPC)]
        adst_tiles = [persist.tile([128, H], f32, name=f"adst{k}") for k in range(TPC)]

        def phase1_emit(l, k):
            ps1 = pspool.tile([128, F + H], f32, name="ps1", tag="ps1")
            nc.tensor.matmul(ps1[:], xT_tiles[k][:], wcat_sb[l][:],
                             start=True, stop=True)
            h_sb = spool.tile([128, F], bf, name="h_sb", tag="h_sb")
            nc.vector.tensor_copy(out=h_sb[:], in_=ps1[:, :F])
            nc.vector.tensor_copy(out=adst_tiles[k][:], in_=ps1[:, F:F + H])
            nc.sync.dma_start(out=hslice[k * 128:(k + 1) * 128, :], in_=h_sb[:])

        def chunk_collective(tbl, j):
            nc.gpsimd.collective_compute(
                "AllGather", ALU.bypass,
                replica_groups=[list(range(NC))],
                ins=[hslice[j * CHT * 128:(j + 1) * CHT * 128, :].opt()],
                outs=[tbl[128 + j * NC * CHT * 128:
                          128 + (j + 1) * NC * CHT * 128, :].opt()])

        def slot_emit(l, k, tbl):
            ka, kb = int(K_A[k]), int(K_B[k])
            kk = ka + kb
            G = gpool.tile([128, kk, F], bf, name="G", tag="G")
            for (s0, n, isb) in pieces[k]:
                nc.gpsimd.reg_mov(nidx_reg, 128 * n)
                nc.gpsimd.dma_gather(
                    out_ap=G[:, s0:s0 + n, :],
                    in_ap=tbl[SPLIT:] if isb else tbl[:],
                    idxs_ap=gidx_sb[:, oi[k] + 8 * s0: oi[k] + 8 * (s0 + n)],
                    num_idxs=128 * n, num_idxs_reg=nidx_reg, elem_size=F,
                    single_packet=False)

            base = G[:, :, :]
            g_ch0 = _ap_view(base, [(F, kk), (CH, H)])       # [128,kk,H]
            e_f = spool.tile([128, kk, H], f32, name="e_f", tag="e_f")
            ad_b = _ap_view(adst_tiles[k][:], [(0, kk), (1, H)])
            nc.vector.tensor_tensor(out=e_f[:], in0=g_ch0, in1=ad_b,
                                    op=ALU.add)
            # leaky relu on DVE: e = max(e, 0.2e)
            e2 = spool.tile([128, kk, H], f32, name="e2", tag="e2")
            nc.vector.tensor_scalar_mul(e2[:], e_f[:], 0.2)
            nc.vector.tensor_tensor(out=e_f[:], in0=e_f[:], in1=e2[:],
                                    op=ALU.max)
            # w expanded over channels on ACT (keeps big mult flat/2x)
            wex = wxpool.tile([128, kk, F], bf, name="wex", tag="wex")
            e_bc = _ap_view(e_f[:], [(H, kk), (1, H), (0, CH)])
            nc.scalar.activation(out=wex[:], in_=e_bc, func=ACTF.Exp,
                                 bias=negc[:], scale=1.0)
            # s = sum_j w  -> [128, H]
            s_f = spool.tile([128, H], f32, name="s_f", tag="s_f")
            w_hj = _ap_view(wex[:, :, :], [(CH, H), (F, kk)])
            nc.vector.tensor_reduce(out=s_f[:], in_=w_hj, axis=AX.X,
                                    op=ALU.add)
            nc.vector.tensor_scalar_add(s_f[:], s_f[:], 1e-16)
            rs_f = spool.tile([128, H], f32, name="rs_f", tag="rs_f")
            nc.vector.reciprocal(out=rs_f[:], in_=s_f[:])
            # M = G * w, flat 2x
            M = mpool.tile([128, kk, F], bf, name="M", tag="M")
            nc.vector.tensor_tensor(
                out=_ap_view(M[:, :, :], [(1, kk * F)]),
                in0=_ap_view(base, [(1, kk * F)]),
                in1=_ap_view(wex[:, :, :], [(1, kk * F)]), op=ALU.mult)
            # single f32-accum reduce over slots
            orot = spool.tile([128, F], f32, name="orot", tag="orot")
            m_red = _ap_view(M[:, :, :], [(CH, H), (1, CH), (F, kk)])
            nc.vector.tensor_reduce(out=orot[:], in_=m_red, axis=AX.X,
                                    op=ALU.add)
            # normalize by 1/s, cast bf16
            on_bf = spool.tile([128, F], bf, name="on_bf", tag="on_bf")
            o_3d = _ap_view(orot[:], [(CH, H), (1, CH)])
            on_3d = _ap_view(on_bf[:], [(CH, H), (1, CH)])
            rs_b = _ap_view(rs_f[:], [(1, H), (0, CH)])
            nc.vector.tensor_tensor(out=on_3d, in0=o_3d, in1=rs_b,
                                    op=ALU.mult)
            # transpose, un-rotate with bmat stationary -> lands feature-major
            ps_t = pspool2.tile([128, 128], bf, name="ps_t", tag="ps_t")
            nc.tensor.transpose(out=ps_t[:], in_=on_bf[:], identity=ident[:])
            onT = spool.tile([128, F], bf, name="onT", tag="on_bf")
            nc.vector.tensor_copy(out=onT[:], in_=ps_t[:])
            ps_x = pspool2.tile([128, F], f32, name="ps_x", tag="ps_x")
            nc.tensor.matmul(ps_x[:], bmat_sb[l][:], onT[:],
                             start=True, stop=True)
            nc.scalar.activation(out=xT_tiles[k][:], in_=ps_x[:],
                                 func=ACTF.Relu, bias=zero_b[:], scale=1.0)

        for rep in range(repeats):
            # (re)load layer-0 xT (bf16, feat-major) per tile + phase 1
            for k in range(TPC):
                xf = spool.tile([128, 128], f32, name="xf", tag="xf")
                nc.sync.dma_start(out=xf[:], in_=xT_p[:, k * 128:(k + 1) * 128])
                nc.vector.tensor_copy(out=xT_tiles[k][:], in_=xf[:])
                phase1_emit(0, k)
                if k % CHT == CHT - 1:
                    chunk_collective(tables[0], k // CHT)
            for l in range(L):
                tbl = tables[l]
                for k in range(TPC):
                    slot_emit(l, k, tbl)
                    if l + 1 < L:
                        phase1_emit(l + 1, k)
                        if k % CHT == CHT - 1:
                            chunk_collective(tables[l + 1], k // CHT)

        # ---- final projection ----
        for k in range(TPC):
            ps_o = pspool.tile([128, FOUT], f32, name="ps_o", tag="ps1")
            nc.tensor.matmul(ps_o[:], xT_tiles[k][:], wfT_sb[:],
                             start=True, stop=True)
            o_sb = spool.tile([128, FOUT], f32, name="o_sb", tag="o_sb")
            nc.vector.tensor_copy(out=o_sb[:], in_=ps_o[:])
            nc.sync.dma_start(out=out_p[k * 128:(k + 1) * 128, :], in_=o_sb[:])

    nc.finalize()
    return nc


TIME_ITERS = 0
LAST_TIMES = None
PROFILE_HW = False
LAST_HW_EXEC_NS = None


def _run_pjrt(nc, in_maps):
    """Execute the Bass module via PJRT (axon).  Mirrors
    bass2jax.run_bass_via_pjrt but keeps the jitted callable so repeated
    executions can be timed on pre-staged device buffers."""
    import jax
    import time
    from jax.sharding import Mesh, PartitionSpec
    from jax.experimental.shard_map import shard_map
    from concourse import bass2jax, mybir
    from concourse.bass2jax import _bass_exec_p, install_neuronx_cc_hook

    install_neuronx_cc_hook()
    n_cores = len(in_maps)
    partition_name = (nc.partition_id_tensor.name
                      if nc.partition_id_tensor else None)
    in_names, out_names, out_avals, zero_outs = [], [], [], []
    for alloc in nc.m.functions[0].allocations:
        if not isinstance(alloc, mybir.MemoryLocationSet):
            continue
        name = alloc.memorylocations[0].name
        if alloc.kind == "ExternalInput":
            if name != partition_name:
                in_names.append(name)
        elif alloc.kind == "ExternalOutput":
            out_names.append(name)
            shape = tuple(alloc.tensor_shape)
            dtype = mybir.dt.np(alloc.dtype)
            out_avals.append(jax.core.ShapedArray(shape, dtype))
            zero_outs.append(np.zeros(shape, dtype))
    n_params = len(in_names)
    n_outs = len(out_avals)
    in_names.extend(out_names)
    if partition_name is not None:
        in_names.append(partition_name)
    donate = tuple(range(n_params, n_params + n_outs))

    def _body(*args):
        operands = list(args)
        if partition_name is not None:
            operands.append(bass2jax.partition_id_tensor())
        outs = _bass_exec_p.bind(
            *operands, out_avals=tuple(out_avals), in_names=tuple(in_names),
            out_names=tuple(out_names), lowering_input_output_aliases=(),
            sim_require_finite=True, sim_require_nnan=True, nc=nc)
        return tuple(outs)

    devices = jax.devices()[:n_cores]
    mesh = Mesh(np.asarray(devices), ("core",))
    in_specs = (PartitionSpec("core"),) * (n_params + n_outs)
    out_specs = (PartitionSpec("core"),) * len(out_names)
    sharded = jax.jit(
        shard_map(_body, mesh=mesh, in_specs=in_specs, out_specs=out_specs,
                  check_rep=False),
        donate_argnums=donate, keep_unused=True)
    per_core = [[np.asarray(m[name]) for name in in_names[:n_params]]
                for m in in_maps]
    concat_in = [np.concatenate([per_core[c][i] for c in range(n_cores)], axis=0)
                 for i in range(n_params)]
    concat_zeros = [np.zeros((n_cores * z.shape[0], *z.shape[1:]), z.dtype)
                    for z in zero_outs]
    out_arrs = sharded(*concat_in, *concat_zeros)
    results = [
        {name: np.asarray(out_arrs[i]).reshape(n_cores, *out_avals[i].shape)[c]
         for i, name in enumerate(out_names)}
        for c in range(n_cores)]

    global LAST_TIMES
    LAST_TIMES = None
    if TIME_ITERS > 0:
        from jax.sharding import NamedSharding
        shardings = [NamedSharding(mesh, PartitionSpec("core"))] * n_params
        dev_in = jax.device_put(concat_in, shardings)
        times = []
        for _ in range(TIME_ITERS):
            zz = jax.device_put(
                concat_zeros,
                [NamedSharding(mesh, PartitionSpec("core"))] * n_outs)
            jax.block_until_ready(zz)
            t0 = time.perf_counter()
            o = sharded(*dev_in, *zz)
            jax.block_until_ready(o)
            times.append(time.perf_counter() - t0)
        LAST_TIMES = times
    return results


def _profile_hw(nc, in_maps):
    """One traced execution via the axon NTFF hook; returns exec_time_ns
    (real on-device time from neuron-profile) or None if unavailable."""
    import os
    import sys
    import types
    import tempfile
    try:
        try:
            from antenv.axon_hooks import get_axon_ntff_profile_hook  # noqa
        except ImportError:
            mod = types.ModuleType("antenv.axon_hooks")
            mod._hook = None
            mod.set_axon_ntff_profile_hook = lambda h: setattr(mod, "_hook", h)
            mod.get_axon_ntff_profile_hook = lambda: mod._hook
            sys.modules["antenv.axon_hooks"] = mod
            import antenv
            antenv.axon_hooks = mod
            if "/root/.axon_site" not in sys.path:
                sys.path.insert(0, "/root/.axon_site")
            from trn_agent_boot.trn_boot import _ntff_profile_via_ctypes
            mod._hook = _ntff_profile_via_ctypes("/opt/axon/libaxon_pjrt.so")
        import concourse.bass_utils as bu
        bu.upload_artifacts = lambda tmpdir: "local://" + tmpdir
        res = bu.run_bass_kernel_spmd(
            nc, in_maps, core_ids=list(range(NC)), trace=True,
            tmpdir=tempfile.mkdtemp(prefix="ntff_"))
        return res.exec_time_ns
    except Exception as err:  # profiling is best-effort
        print("HW profile failed:", err)
        return None


def _make_in_maps(pp):
    in_maps = []
    for c in range(NC):
        rows = _rows_of_core(c)
        in_maps.append({
            "xT": np.ascontiguousarray(pp["x_dev"][rows].T).astype(np.float32),
            "gidx": np.ascontiguousarray(pp["gidx"][c]),
            "wcat": pp["wcat"],
            "bmat": pp["bmat"],
            "wfT": pp["wfT"],
        })
    return in_maps


def kernel(**inputs):
    import sys
    if "/opt/trn_rl_repo" not in sys.path:
        sys.path.insert(0, "/opt/trn_rl_repo")

    inputs = {k: np.asarray(v) for k, v in inputs.items()}
    pp = _host_prep(inputs)
    nc = _build_bass(pp)

    in_maps = _make_in_maps(pp)
    results = _run_pjrt(nc, in_maps)
    global LAST_HW_EXEC_NS
    LAST_HW_EXEC_NS = None
    if PROFILE_HW:
        LAST_HW_EXEC_NS = _profile_hw(nc, in_maps)
    out_dev = np.zeros((NPAD, FOUT), np.float32)
    for c in range(NC):
        out_dev[_rows_of_core(c)] = results[c]["out"]
    return out_dev[pp["rowof"]].astype(np.float32)


if __name__ == "__main__":
    pass
